# revision 2
# baseline (speedup 1.0000x reference)
"""Tensor-parallel multi-head attention for 8 Trainium2 NeuronCores.

Sharding (DP2 x TP4): cores 0-3 take batch 0, cores 4-7 batch 1. Within a
group of 4 cores, heads are split 4 ways (4 heads = 256 q/k/v features per
core); out_proj is column-sharded with an intra-group AllGather of the
per-core context shards.

Per-core dataflow (activations kept transposed, [feature, token]):
  qT/kT/vT = W.T-chunks @ xT          (PE, bf16, fp32 PSUM)
  v        = PE-transpose(vT)          (with an appended ones-column)
  sT[k,q]  = kT-block.T @ qT           (causal: upper-right blocks skipped)
  aT       = exp(sT/8 + mask_bias)     (ACT; no max-subtraction needed:
                                        scores ~ N(0,1), exp is safe)
  ctxT;sum = [v|1].T @ aT              (ones row gives the softmax denom)
  ctxT     = ctxT * (1/sum)            (DVE + gpsimd partition_broadcast)
  AllGather ctxT within group -> full [1024, 2048] context
  outT     = woT-chunks.T @ ctxT_full  (column shard of out_proj)
Host side only reshapes/concatenates shards (no arithmetic besides dtype
prep of the inputs).
"""

import sys

for _p in ("/opt/trn_rl_repo",):
    if _p not in sys.path:
        sys.path.append(_p)

import numpy as np
import ml_dtypes

import concourse.bass as bass  # noqa: F401
import concourse.mybir as mybir
import concourse.tile as tile
from concourse import bacc, bass_utils
from concourse.masks import make_identity, make_upper_triangular

BF16 = mybir.dt.bfloat16
F32 = mybir.dt.float32
Exp = mybir.ActivationFunctionType.Exp

B, S, D = 2, 2048, 1024
H, DH = 16, 64
NCORES = 8
TPG = 4              # tensor-parallel group size (cores per batch)
HPC = H // TPG       # heads per core = 4
F = HPC * DH         # features per core = 256
KC = S // 128        # 16 k-chunks of 128
QB = S // 512        # 4 q-blocks of 512

_CACHED = {}


def _build():
    nc = bacc.Bacc(
        "TRN2",
        target_bir_lowering=False,
        debug=False,
        enable_asserts=True,
        num_devices=NCORES,
    )
    xT_d = nc.dram_tensor("xT", [D, S], BF16, kind="ExternalInput").ap()
    wqT_d = nc.dram_tensor("wqT", [D, F], BF16, kind="ExternalInput").ap()
    wkT_d = nc.dram_tensor("wkT", [D, F], BF16, kind="ExternalInput").ap()
    wvT_d = nc.dram_tensor("wvT", [D, F], BF16, kind="ExternalInput").ap()
    woT_d = nc.dram_tensor("woT", [D, F], BF16, kind="ExternalInput").ap()
    bq_d = nc.dram_tensor("bq", [1, F], BF16, kind="ExternalInput").ap()
    bk_d = nc.dram_tensor("bk", [1, F], BF16, kind="ExternalInput").ap()
    bv_d = nc.dram_tensor("bv", [1, F], BF16, kind="ExternalInput").ap()
    bo_d = nc.dram_tensor("bo", [1, F], BF16, kind="ExternalInput").ap()
    maskb_d = nc.dram_tensor("maskb", [128, KC], F32, kind="ExternalInput").ap()
    outT_d = nc.dram_tensor("outT", [F, S], F32, kind="ExternalOutput").ap()

    with tile.TileContext(nc) as tc:
        with (
            tc.tile_pool(name="singles", bufs=1) as sg,
            tc.tile_pool(name="att", bufs=3) as att_pool,
            tc.tile_pool(name="psA", bufs=2, space="PSUM") as psA,
            tc.tile_pool(name="psB", bufs=4, space="PSUM") as psB,
            tc.tile_pool(name="dram", bufs=1, space="DRAM") as dram,
        ):
            # ---- constants -------------------------------------------------
            ident = sg.tile([128, 128], BF16, name="ident")
            make_identity(nc, ident)
            trimask = sg.tile([128, 128], BF16, name="trimask")
            make_upper_triangular(nc, trimask, val=1.0, diag=True)
            ones512 = sg.tile([1, 512], BF16, name="ones512")
            nc.vector.memset(ones512, 1.0)

            # ---- load inputs ----------------------------------------------
            xT_sb = sg.tile([128, 8, S], BF16, name="xT_sb")
            nc.sync.dma_start(xT_sb, xT_d.rearrange("(o p) f -> p o f", p=128))
            w_sb = {}
            for nm, d in (("q", wqT_d), ("k", wkT_d), ("v", wvT_d), ("o", woT_d)):
                w_sb[nm] = sg.tile([128, 8, F], BF16, name=f"w{nm}T_sb")
                nc.sync.dma_start(w_sb[nm], d.rearrange("(o p) f -> p o f", p=128))
            b_sb = {}
            for nm, d in (("q", bq_d), ("k", bk_d), ("v", bv_d), ("o", bo_d)):
                b_sb[nm] = sg.tile([1, F], BF16, name=f"b{nm}_sb")
                nc.sync.dma_start(b_sb[nm], d)
            maskb_sb = sg.tile([128, KC], F32, name="maskb_sb")
            nc.sync.dma_start(maskb_sb, maskb_d)

            # ---- projections ----------------------------------------------
            qT_sb = sg.tile([128, 2, S], BF16, name="qT_sb")
            kT_sb = sg.tile([128, 2, S], BF16, name="kT_sb")
            vT_sb = sg.tile([128, 2, S], BF16, name="vT_sb")

            def project(w, bias, dst, which):
                for mo in range(2):
                    for half in range(2):
                        ps = psA.tile(
                            [128, 1024], F32, tag="work",
                            name=f"p_{which}_{mo}_{half}",
                        )
                        for nb in range(2):
                            cs = half * 1024 + nb * 512
                            for ki in range(8):
                                nc.tensor.matmul(
                                    ps[:, nb * 512:nb * 512 + 512],
                                    lhsT=w[:, ki, mo * 128:mo * 128 + 128],
                                    rhs=xT_sb[:, ki, cs:cs + 512],
                                    start=(ki == 0),
                                    stop=False,
                                )
                            nc.tensor.matmul(
                                ps[:, nb * 512:nb * 512 + 512],
                                lhsT=bias[0:1, mo * 128:mo * 128 + 128],
                                rhs=ones512[0:1, :],
                                start=False,
                                stop=True,
                            )
                        nc.vector.tensor_copy(
                            dst[:, mo, half * 1024:half * 1024 + 1024], ps
                        )

            project(w_sb["v"], b_sb["v"], vT_sb, "v")
            project(w_sb["k"], b_sb["k"], kT_sb, "k")
            project(w_sb["q"], b_sb["q"], qT_sb, "q")

            # ---- transpose v into [token, feat] blocks with ones column ----
            v_ones = sg.tile([128, KC, HPC, DH + 1], BF16, name="v_ones")
            nc.vector.memset(v_ones, 1.0)
            for mo in range(2):
                for tb in range(KC):
                    pt = psB.tile([128, 128], BF16, tag="ctx", name=f"vt_{mo}_{tb}")
                    nc.tensor.transpose(
                        pt, vT_sb[:, mo, tb * 128:tb * 128 + 128], ident
                    )
                    for hh in range(2):
                        h = mo * 2 + hh
                        nc.vector.tensor_copy(
                            v_ones[:, tb, h, 0:DH], pt[:, hh * 64:hh * 64 + 64]
                        )

            # ---- attention per head ---------------------------------------
            ctxT_sb = sg.tile([64, HPC, S], BF16, name="ctxT_sb")
            sums_sb = sg.tile([1, S], F32, name="sums_sb")
            recip_sb = sg.tile([1, S], F32, name="recip_sb")
            recip_bc = sg.tile([64, S], F32, name="recip_bc")

            for h in range(HPC):
                mo, po = h // 2, 64 * (h % 2)
                ctx_ps = [
                    psB.tile([128, 512], F32, tag="ctx", name=f"ctx_{h}_{qb}")
                    for qb in range(QB)
                ]
                for kc in range(KC):
                    q0 = kc * 128
                    kT_blk = kT_sb[po:po + 64, mo, kc * 128:kc * 128 + 128]
                    halves = [hb for hb in (0, 1024) if hb + 1024 > q0]
                    for hb in halves:
                        lo = max(q0, hb)
                        hi = hb + 1024
                        st = psA.tile(
                            [128, 1024], F32, tag="work",
                            name=f"st_{h}_{kc}_{hb}",
                        )
                        c = lo
                        while c < hi:
                            c2 = min(hi, (c // 512 + 1) * 512)
                            nc.tensor.matmul(
                                st[:, c - hb:c2 - hb],
                                lhsT=kT_blk,
                                rhs=qT_sb[po:po + 64, mo, c:c2],
                                start=True,
                                stop=True,
                            )
                            c = c2
                        at = att_pool.tile([128, 1024], BF16, tag="att")
                        nc.scalar.activation(
                            at[:, lo - hb:hi - hb],
                            st[:, lo - hb:hi - hb],
                            Exp,
                            bias=maskb_sb[:, kc:kc + 1],
                            scale=0.125,
                        )
                        if lo == q0:  # diagonal 128-block: causal interior
                            nc.vector.tensor_mul(
                                at[:, q0 - hb:q0 - hb + 128],
                                at[:, q0 - hb:q0 - hb + 128],
                                trimask,
                            )
                        c = lo
                        while c < hi:
                            qb = c // 512
                            c2 = min(hi, (qb + 1) * 512)
                            nc.tensor.matmul(
                                ctx_ps[qb][0:DH + 1, c - qb * 512:c2 - qb * 512],
                                lhsT=v_ones[:, kc, h, :],
                                rhs=at[:, c - hb:c2 - hb],
                                start=(kc == 0),
                                stop=(kc == 4 * qb + 3),
                            )
                            c = c2
                # normalize: rows 0..63 / row 64
                for qb in range(QB):
                    nc.vector.tensor_copy(
                        sums_sb[0:1, qb * 512:qb * 512 + 512],
                        ctx_ps[qb][DH:DH + 1, :],
                    )
                nc.vector.reciprocal(recip_sb, sums_sb)
                nc.gpsimd.partition_broadcast(recip_bc, recip_sb)
                for qb in range(QB):
                    nc.vector.tensor_mul(
                        ctxT_sb[:, h, qb * 512:qb * 512 + 512],
                        ctx_ps[qb][0:DH, :],
                        recip_bc[:, qb * 512:qb * 512 + 512],
                    )

            # ---- AllGather context within group ---------------------------
            cc_in = dram.tile([F, S], BF16, name="cc_in")
            # NB: Shared-output collectives need >4-core groups; Local it is.
            cc_out = dram.tile([TPG * F, S], BF16, name="cc_out")
            nc.sync.dma_start(
                cc_in.rearrange("(h d) f -> d h f", d=64), ctxT_sb
            )
            nc.gpsimd.collective_compute(
                "AllGather",
                mybir.AluOpType.bypass,
                replica_groups=[[0, 1, 2, 3], [4, 5, 6, 7]],
                ins=[cc_in.opt()],
                outs=[cc_out.opt()],
            )
            ctxF_sb = sg.tile([128, 8, S], BF16, name="ctxF_sb")
            cc_out_r = cc_out.rearrange("(o p) f -> p o f", p=128)
            for ki in range(8):
                nc.sync.dma_start(ctxF_sb[:, ki, :], cc_out_r[:, ki, :])

            # ---- out projection (column shard) ----------------------------
            outT_sb = sg.tile([128, 2, S], F32, name="outT_sb")
            for mo in range(2):
                for half in range(2):
                    ps = psA.tile(
                        [128, 1024], F32, tag="work", name=f"o_{mo}_{half}"
                    )
                    for nb in range(2):
                        cs = half * 1024 + nb * 512
                        for ki in range(8):
                            nc.tensor.matmul(
                                ps[:, nb * 512:nb * 512 + 512],
                                lhsT=w_sb["o"][:, ki, mo * 128:mo * 128 + 128],
                                rhs=ctxF_sb[:, ki, cs:cs + 512],
                                start=(ki == 0),
                                stop=False,
                            )
                        nc.tensor.matmul(
                            ps[:, nb * 512:nb * 512 + 512],
                            lhsT=b_sb["o"][0:1, mo * 128:mo * 128 + 128],
                            rhs=ones512[0:1, :],
                            start=False,
                            stop=True,
                        )
                    nc.vector.tensor_copy(
                        outT_sb[:, mo, half * 1024:half * 1024 + 1024], ps
                    )
            nc.sync.dma_start(
                outT_d.rearrange("(o p) f -> p o f", p=128), outT_sb
            )

    nc.compile()
    return nc


def _get_program():
    if "nc" not in _CACHED:
        _CACHED["nc"] = _build()
    return _CACHED["nc"]


def kernel(x, mask, wq, bq, wk, bk, wv, bv, wo, bo):
    x = np.asarray(x, dtype=np.float32)
    mask = np.asarray(mask)
    bf = ml_dtypes.bfloat16

    nc = _get_program()

    xT = [np.ascontiguousarray(x[g].T).astype(bf) for g in range(B)]
    maskb = [
        np.ascontiguousarray(
            np.where(mask[g], -10000.0, 0.0).astype(np.float32).reshape(KC, 128).T
        )
        for g in range(B)
    ]
    in_maps = []
    for c in range(NCORES):
        g, t = c // TPG, c % TPG
        fs = slice(t * F, (t + 1) * F)
        in_maps.append(
            {
                "xT": xT[g],
                "wqT": np.ascontiguousarray(wq[fs, :].T).astype(bf),
                "wkT": np.ascontiguousarray(wk[fs, :].T).astype(bf),
                "wvT": np.ascontiguousarray(wv[fs, :].T).astype(bf),
                "woT": np.ascontiguousarray(wo[fs, :].T).astype(bf),
                "bq": np.asarray(bq[fs], dtype=bf).reshape(1, F),
                "bk": np.asarray(bk[fs], dtype=bf).reshape(1, F),
                "bv": np.asarray(bv[fs], dtype=bf).reshape(1, F),
                "bo": np.asarray(bo[fs], dtype=bf).reshape(1, F),
                "maskb": maskb[g],
            }
        )

    res = bass_utils.run_bass_kernel_spmd(
        nc, in_maps, core_ids=list(range(NCORES)), trace=False
    )
    _CACHED["last_results"] = res

    out = np.empty((B, S, D), dtype=np.float32)
    for c in range(NCORES):
        g, t = c // TPG, c % TPG
        out[g, :, t * F:(t + 1) * F] = res.results[c]["outT"].T
    return out


# revision 13
# speedup vs baseline: 1.0002x; 1.0002x over previous
"""Tensor-parallel multi-head attention for 8 Trainium2 NeuronCores.

Sharding (TP8 over heads): core c owns heads {2c, 2c+1} (128 q/k/v features)
and computes them for BOTH batch elements; out_proj is column-sharded with an
8-core mesh AllGather of the per-core context shards (8-core mesh is ~4x
faster than a 4-core ring at these sizes, and keeps the gathered layout
core-independent).

Per-core dataflow (activations kept transposed, [feature, token]):
  qT/kT/vT = W.T-chunks @ xT          (PE, bf16, fp32 PSUM accum)
  v        = PE-transpose(vT)          (with an appended ones-column)
  sT[k,q]  = kT-block.T @ qT           (causal: upper-right blocks skipped)
  aT       = exp(sT/8 + mask_bias)     (ACT; safe without max-subtraction:
                                        scores ~ N(0,1))
  ctxT;sum = [v|1].T @ aT              (ones row gives the softmax denom)
  ctxT    *= 1/sum                     (fp32r ones-bcast matmul spreads the
                                        denom over 64 partitions, then DVE
                                        reciprocal + multiply)
  AllGather ctxT across all 8 cores -> full [1024, 2*2048] context
  outT     = woT-chunks.T @ ctxT_full  (column shard of out_proj)
Host side only reshapes/concatenates shards (dtype prep of inputs aside).
"""

import sys

for _p in ("/opt/trn_rl_repo",):
    if _p not in sys.path:
        sys.path.append(_p)

import numpy as np
import ml_dtypes

import concourse.bass as bass  # noqa: F401
import concourse.mybir as mybir
import concourse.tile as tile
from concourse import bacc, bass_utils
from concourse.masks import make_identity, make_upper_triangular

BF16 = mybir.dt.bfloat16
F32 = mybir.dt.float32
F32R = mybir.dt.float32r
Exp = mybir.ActivationFunctionType.Exp

B, S, D = 2, 2048, 1024
T = B * S            # 4096 tokens across batches
H, DH = 16, 64
NCORES = 8
HPC = H // NCORES    # heads per core = 2
F = HPC * DH         # features per core = 128
KC = S // 128        # 16 k-chunks per batch
QB = S // 512        # 4 q-blocks per batch

_CACHED = {}


def _build(with_bias: bool):
    nc = bacc.Bacc(
        "TRN2",
        target_bir_lowering=False,
        debug=False,
        enable_asserts=True,
        num_devices=NCORES,
    )
    xT_d = nc.dram_tensor("xT", [D, T], BF16, kind="ExternalInput").ap()
    wqT_d = nc.dram_tensor("wqT", [D, F], BF16, kind="ExternalInput").ap()
    wkT_d = nc.dram_tensor("wkT", [D, F], BF16, kind="ExternalInput").ap()
    wvT_d = nc.dram_tensor("wvT", [D, F], BF16, kind="ExternalInput").ap()
    woT_d = nc.dram_tensor("woT", [D, F], BF16, kind="ExternalInput").ap()
    b_d = {}
    if with_bias:
        for nm in ("bq", "bk", "bv", "bo"):
            b_d[nm] = nc.dram_tensor(nm, [1, F], BF16, kind="ExternalInput").ap()
    maskb_d = nc.dram_tensor("maskb", [128, B * KC], F32, kind="ExternalInput").ap()
    outT_d = nc.dram_tensor("outT", [F, T], F32, kind="ExternalOutput").ap()

    with tile.TileContext(nc) as tc:
        with (
            tc.tile_pool(name="singles", bufs=1) as sg,
            tc.tile_pool(name="att", bufs=3) as att_pool,
            tc.tile_pool(name="psA", bufs=2, space="PSUM") as psA,
            tc.tile_pool(name="psB", bufs=4, space="PSUM") as psB,
            tc.tile_pool(name="dram", bufs=1, space="DRAM") as dram,
        ):
            # ---- constants -------------------------------------------------
            ident = sg.tile([128, 128], BF16, name="ident")
            make_identity(nc, ident)
            trimask = sg.tile([128, 128], BF16, name="trimask")
            make_upper_triangular(nc, trimask, val=1.0, diag=True)
            ones64f = sg.tile([1, 64], F32, name="ones64f")
            nc.vector.memset(ones64f, 1.0)
            ones64r = sg.tile([1, 64], F32R, name="ones64r")
            nc.vector.tensor_copy(ones64r, ones64f)
            if with_bias:
                ones512 = sg.tile([1, 512], BF16, name="ones512")
                nc.vector.memset(ones512, 1.0)

            # ---- load inputs (split for early start) -----------------------
            maskb_sb = sg.tile([128, B * KC], F32, name="maskb_sb")
            nc.sync.dma_start(maskb_sb, maskb_d)
            w_sb = {}
            for nm, dd in (("v", wvT_d), ("k", wkT_d), ("q", wqT_d), ("o", woT_d)):
                w_sb[nm] = sg.tile([128, 8, F], BF16, name=f"w{nm}T_sb")
                nc.sync.dma_start(w_sb[nm], dd.rearrange("(o p) f -> p o f", p=128))
            b_sb = {}
            if with_bias:
                for nm in ("bq", "bk", "bv", "bo"):
                    b_sb[nm] = sg.tile([1, F], BF16, name=f"{nm}_sb")
                    nc.sync.dma_start(b_sb[nm], b_d[nm])
            xT_sb, xT_free = tc.tile([128, 8, T], BF16, name="xT_sb")
            xT_r = xT_d.rearrange("(o p) f -> p o f", p=128)
            for ki in range(8):
                nc.sync.dma_start(xT_sb[:, ki, :], xT_r[:, ki, :])

            # ---- projections ----------------------------------------------
            qT_sb, qT_free = tc.tile([128, T], BF16, name="qT_sb")
            kT_sb, kT_free = tc.tile([128, T], BF16, name="kT_sb")
            vT_sb, vT_free = tc.tile([128, T], BF16, name="vT_sb")

            def project(w, bias, dst, which):
                for half in range(4):
                    ps = psA.tile(
                        [128, 1024], F32, tag="work", name=f"p_{which}_{half}"
                    )
                    for nb in range(2):
                        cs = half * 1024 + nb * 512
                        for ki in range(8):
                            nc.tensor.matmul(
                                ps[:, nb * 512:nb * 512 + 512],
                                lhsT=w[:, ki, :],
                                rhs=xT_sb[:, ki, cs:cs + 512],
                                start=(ki == 0),
                                stop=(ki == 7 and not with_bias),
                            )
                        if with_bias:
                            nc.tensor.matmul(
                                ps[:, nb * 512:nb * 512 + 512],
                                lhsT=bias[0:1, :],
                                rhs=ones512[0:1, :],
                                start=False,
                                stop=True,
                            )
                    nc.vector.tensor_copy(
                        dst[:, half * 1024:half * 1024 + 1024], ps
                    )

            project(w_sb["v"], b_sb.get("bv"), vT_sb, "v")
            project(w_sb["k"], b_sb.get("bk"), kT_sb, "k")
            project(w_sb["q"], b_sb.get("bq"), qT_sb, "q")

            # ---- transpose v into [token, feat] blocks with ones column ----
            v_ones = sg.tile([128, B * KC, HPC, DH + 1], BF16, name="v_ones")
            nc.vector.memset(v_ones, 1.0)
            for tb in range(B * KC):
                pt = psB.tile([128, 128], BF16, tag="ctx", name=f"vt_{tb}")
                nc.tensor.transpose(pt, vT_sb[:, tb * 128:tb * 128 + 128], ident)
                for h in range(HPC):
                    nc.vector.tensor_copy(
                        v_ones[:, tb, h, 0:DH], pt[:, h * 64:h * 64 + 64]
                    )

            # ---- attention per (batch, head) ------------------------------
            ctxT_sb, ctxT_free = tc.tile([64, HPC, T], BF16, name="ctxT_sb")
            sums_r = sg.tile([1, 1024], F32R, name="sums_r")
            rec_sb = sg.tile([64, 1024], F32, name="rec_sb")

            for b in range(B):
                t0 = b * S
                for h in range(HPC):
                    po = 64 * h
                    ctx_ps = [
                        psB.tile([128, 512], F32, tag="ctx", name=f"ctx_{b}_{h}_{qb}")
                        for qb in range(QB)
                    ]
                    for kc in range(KC):
                        q0 = kc * 128  # batch-local
                        kT_blk = kT_sb[po:po + 64, t0 + q0:t0 + q0 + 128]
                        for hb in (0, 1024):
                            if hb + 1024 <= q0:
                                continue
                            lo = max(q0, hb)
                            hi = hb + 1024
                            st = psA.tile(
                                [128, 1024], F32, tag="work",
                                name=f"st_{b}_{h}_{kc}_{hb}",
                            )
                            c = lo
                            while c < hi:
                                c2 = min(hi, (c // 512 + 1) * 512)
                                nc.tensor.matmul(
                                    st[:, c - hb:c2 - hb],
                                    lhsT=kT_blk,
                                    rhs=qT_sb[po:po + 64, t0 + c:t0 + c2],
                                    start=True,
                                    stop=True,
                                )
                                c = c2
                            at = att_pool.tile([128, 1024], BF16, tag="att")
                            nc.scalar.activation(
                                at[:, lo - hb:hi - hb],
                                st[:, lo - hb:hi - hb],
                                Exp,
                                bias=maskb_sb[:, b * KC + kc:b * KC + kc + 1],
                                scale=0.125,
                            )
                            if lo == q0:  # diagonal 128-block
                                nc.vector.tensor_mul(
                                    at[:, q0 - hb:q0 - hb + 128],
                                    at[:, q0 - hb:q0 - hb + 128],
                                    trimask,
                                )
                            c = lo
                            while c < hi:
                                qb = c // 512
                                c2 = min(hi, (qb + 1) * 512)
                                nc.tensor.matmul(
                                    ctx_ps[qb][0:DH + 1, c - qb * 512:c2 - qb * 512],
                                    lhsT=v_ones[:, b * KC + kc, h, :],
                                    rhs=at[:, c - hb:c2 - hb],
                                    start=(kc == 0),
                                    stop=(kc == 4 * qb + 3),
                                )
                                c = c2
                    # normalize: rows 0..63 / row 64 (the ones-row sums)
                    for pair in range(2):
                        bc = psA.tile(
                            [128, 1024], F32, tag="work", name=f"bc_{b}_{h}_{pair}"
                        )
                        for j in range(2):
                            qb = 2 * pair + j
                            nc.vector.tensor_copy(
                                sums_r[0:1, j * 512:j * 512 + 512],
                                ctx_ps[qb][DH:DH + 1, :],
                            )
                            nc.tensor.matmul(
                                bc[0:64, j * 512:j * 512 + 512],
                                lhsT=ones64r[0:1, :],
                                rhs=sums_r[0:1, j * 512:j * 512 + 512],
                                start=True,
                                stop=True,
                            )
                        nc.vector.reciprocal(rec_sb, bc[0:64, :])
                        for j in range(2):
                            qb = 2 * pair + j
                            nc.vector.tensor_mul(
                                ctxT_sb[:, h, t0 + qb * 512:t0 + qb * 512 + 512],
                                ctx_ps[qb][0:DH, :],
                                rec_sb[:, j * 512:j * 512 + 512],
                            )

            # ---- AllGather context across all 8 cores ---------------------
            cc_in = dram.tile([F, T], BF16, name="cc_in")
            cc_out = dram.tile(
                [NCORES * F, T], BF16, addr_space="Shared", name="cc_out"
            )
            nc.sync.dma_start(cc_in.rearrange("(h d) f -> d h f", d=64), ctxT_sb)
            nc.gpsimd.collective_compute(
                "AllGather",
                mybir.AluOpType.bypass,
                replica_groups=[list(range(NCORES))],
                ins=[cc_in.opt()],
                outs=[cc_out.opt()],
            )

            # free the big stage-B tiles before staging the gathered context
            # (LIFO: Tile singles are stack-allocated)
            ctxT_free()
            vT_free()
            kT_free()
            qT_free()
            xT_free()

            ctxF_sb, _ctxF_free = tc.tile([128, 8, T], BF16, name="ctxF_sb")
            cc_out_r = cc_out.rearrange("(o p) f -> p o f", p=128)
            for ki in range(8):
                nc.sync.dma_start(ctxF_sb[:, ki, :], cc_out_r[:, ki, :])

            # ---- out projection (column shard) ----------------------------
            outT_sb, _outT_free = tc.tile([128, T], F32, name="outT_sb")
            for half in range(4):
                ps = psA.tile([128, 1024], F32, tag="work", name=f"o_{half}")
                for nb in range(2):
                    cs = half * 1024 + nb * 512
                    for ki in range(8):
                        nc.tensor.matmul(
                            ps[:, nb * 512:nb * 512 + 512],
                            lhsT=w_sb["o"][:, ki, :],
                            rhs=ctxF_sb[:, ki, cs:cs + 512],
                            start=(ki == 0),
                            stop=(ki == 7 and not with_bias),
                        )
                    if with_bias:
                        nc.tensor.matmul(
                            ps[:, nb * 512:nb * 512 + 512],
                            lhsT=b_sb["bo"][0:1, :],
                            rhs=ones512[0:1, :],
                            start=False,
                            stop=True,
                        )
                nc.vector.tensor_copy(
                    outT_sb[:, half * 1024:half * 1024 + 1024], ps
                )
            nc.sync.dma_start(outT_d, outT_sb)
            _outT_free()
            _ctxF_free()

    nc.compile()
    return nc


def _get_program(with_bias: bool = False):
    key = ("nc", with_bias)
    if key not in _CACHED:
        _CACHED[key] = _build(with_bias)
    return _CACHED[key]


def kernel(x, mask, wq, bq, wk, bk, wv, bv, wo, bo):
    x = np.asarray(x, dtype=np.float32)
    mask = np.asarray(mask)
    bf = ml_dtypes.bfloat16

    with_bias = any(
        np.any(np.asarray(bb)) for bb in (bq, bk, bv, bo)
    )
    nc = _get_program(with_bias)

    # [feature, batch*seq] activations
    xT = np.ascontiguousarray(x.reshape(T, D).T).astype(bf)
    maskb = np.ascontiguousarray(
        np.where(np.asarray(mask).reshape(B * KC, 128), -10000.0, 0.0)
        .astype(np.float32)
        .T
    )
    in_maps = []
    for c in range(NCORES):
        fs = slice(c * F, (c + 1) * F)
        m = {
            "xT": xT,
            "wqT": np.ascontiguousarray(np.asarray(wq)[fs, :].T).astype(bf),
            "wkT": np.ascontiguousarray(np.asarray(wk)[fs, :].T).astype(bf),
            "wvT": np.ascontiguousarray(np.asarray(wv)[fs, :].T).astype(bf),
            "woT": np.ascontiguousarray(np.asarray(wo)[fs, :].T).astype(bf),
            "maskb": maskb,
        }
        if with_bias:
            m["bq"] = np.asarray(bq)[fs].astype(bf).reshape(1, F)
            m["bk"] = np.asarray(bk)[fs].astype(bf).reshape(1, F)
            m["bv"] = np.asarray(bv)[fs].astype(bf).reshape(1, F)
            m["bo"] = np.asarray(bo)[fs].astype(bf).reshape(1, F)
        in_maps.append(m)

    res = bass_utils.run_bass_kernel_spmd(
        nc, in_maps, core_ids=list(range(NCORES)), trace=False
    )
    _CACHED["last_results"] = res

    out = np.empty((B, S, D), dtype=np.float32)
    for c in range(NCORES):
        o = res.results[c]["outT"]  # [F, T]
        out[:, :, c * F:(c + 1) * F] = o.T.reshape(B, S, F)
    return out


# revision 14
# speedup vs baseline: 1.2149x; 1.2146x over previous
"""Tensor-parallel multi-head attention for 8 Trainium2 NeuronCores.

Sharding (TP8 over heads): core c owns heads {2c, 2c+1} (128 q/k/v features)
and computes them for BOTH batch elements; out_proj is column-sharded with
8-core mesh AllGathers of the per-core context shards, split 4 ways by
(local head, batch) so the first three overlap the remaining attention work.

Per-core dataflow (activations kept transposed, [feature, token]):
  qT/kT/vT = W.T-chunks @ xT          (PE, bf16, fp32 PSUM accum)
  v        = PE-transpose(vT)          (with an appended ones-column)
  sT[k,q]  = kT-block.T @ qT           (causal: upper-right blocks skipped)
  aT       = exp(sT/8 + mask_bias)     (ACT; safe without max-subtraction:
                                        scores ~ N(0,1))
  ctxT;sum = [v|1].T @ aT              (ones row gives the softmax denom)
  ctxT    *= 1/sum                     (per-q-block, as soon as its k-loop
                                        completes: fp32r ones-bcast matmul
                                        spreads the denom over 64 partitions,
                                        then DVE reciprocal + multiply)
  AllGather ctxT shard per (head,batch) -> full context, out-proj shard
Host side only reshapes/concatenates shards (dtype prep of inputs aside).
The gathered feature order is [local-head, core, dh]; the host permutes
wo's input dimension to match.
"""

import sys

for _p in ("/opt/trn_rl_repo",):
    if _p not in sys.path:
        sys.path.append(_p)

import numpy as np
import ml_dtypes

import concourse.bass as bass  # noqa: F401
import concourse.mybir as mybir
import concourse.tile as tile
from concourse import bacc, bass_utils
from concourse.masks import make_identity, make_upper_triangular

BF16 = mybir.dt.bfloat16
F32 = mybir.dt.float32
F32R = mybir.dt.float32r
Exp = mybir.ActivationFunctionType.Exp

B, S, D = 2, 2048, 1024
T = B * S            # 4096 tokens across batches
H, DH = 16, 64
NCORES = 8
HPC = H // NCORES    # heads per core = 2
F = HPC * DH         # features per core = 128
KC = S // 128        # 16 k-chunks per batch
QB = S // 512        # 4 q-blocks per batch

_CACHED = {}


def _build(with_bias: bool):
    nc = bacc.Bacc(
        "TRN2",
        target_bir_lowering=False,
        debug=False,
        enable_asserts=True,
        num_devices=NCORES,
    )
    xT_d = nc.dram_tensor("xT", [D, T], BF16, kind="ExternalInput").ap()
    wqT_d = nc.dram_tensor("wqT", [D, F], BF16, kind="ExternalInput").ap()
    wkT_d = nc.dram_tensor("wkT", [D, F], BF16, kind="ExternalInput").ap()
    wvT_d = nc.dram_tensor("wvT", [D, F], BF16, kind="ExternalInput").ap()
    woT_d = nc.dram_tensor("woT", [D, F], BF16, kind="ExternalInput").ap()
    b_d = {}
    if with_bias:
        for nm in ("bq", "bk", "bv", "bo"):
            b_d[nm] = nc.dram_tensor(nm, [1, F], BF16, kind="ExternalInput").ap()
    maskb_d = nc.dram_tensor("maskb", [128, B * KC], F32, kind="ExternalInput").ap()
    outT_d = nc.dram_tensor("outT", [F, T], F32, kind="ExternalOutput").ap()

    with tile.TileContext(nc) as tc:
        with (
            tc.tile_pool(name="singles", bufs=1) as sg,
            tc.tile_pool(name="att", bufs=4) as att_pool,
            tc.tile_pool(name="psA", bufs=2, space="PSUM") as psA,
            tc.tile_pool(name="psB", bufs=4, space="PSUM") as psB,
            tc.tile_pool(name="dram", bufs=1, space="DRAM") as dram,
        ):
            # ---- constants -------------------------------------------------
            ident = sg.tile([128, 128], BF16, name="ident")
            make_identity(nc, ident)
            trimask = sg.tile([128, 128], BF16, name="trimask")
            make_upper_triangular(nc, trimask, val=1.0, diag=True)
            ones64f = sg.tile([1, 64], F32, name="ones64f")
            nc.vector.memset(ones64f, 1.0)
            ones64r = sg.tile([1, 64], F32R, name="ones64r")
            nc.vector.tensor_copy(ones64r, ones64f)
            if with_bias:
                ones512 = sg.tile([1, 512], BF16, name="ones512")
                nc.vector.memset(ones512, 1.0)

            # ---- load inputs (split for early start) -----------------------
            maskb_sb = sg.tile([128, B * KC], F32, name="maskb_sb")
            nc.sync.dma_start(maskb_sb, maskb_d)
            w_sb = {}
            for nm, dd in (("v", wvT_d), ("k", wkT_d), ("q", wqT_d), ("o", woT_d)):
                w_sb[nm] = sg.tile([128, 8, F], BF16, name=f"w{nm}T_sb")
                nc.sync.dma_start(w_sb[nm], dd.rearrange("(o p) f -> p o f", p=128))
            b_sb = {}
            if with_bias:
                for nm in ("bq", "bk", "bv", "bo"):
                    b_sb[nm] = sg.tile([1, F], BF16, name=f"{nm}_sb")
                    nc.sync.dma_start(b_sb[nm], b_d[nm])
            xT_sb, xT_free = tc.tile([128, 8, T], BF16, name="xT_sb")
            xT_r = xT_d.rearrange("(o p) f -> p o f", p=128)
            for half in range(4):
                for ki in range(8):
                    cs = half * 1024
                    nc.sync.dma_start(
                        xT_sb[:, ki, cs:cs + 1024], xT_r[:, ki, cs:cs + 1024]
                    )

            # ---- projections ----------------------------------------------
            qT_sb, qT_free = tc.tile([128, T], BF16, name="qT_sb")
            kT_sb, kT_free = tc.tile([128, T], BF16, name="kT_sb")
            vT_sb, vT_free = tc.tile([128, T], BF16, name="vT_sb")

            def project(w, bias, dst, which):
                for half in range(4):
                    ps = psA.tile(
                        [128, 1024], F32, tag="work", name=f"p_{which}_{half}"
                    )
                    for nb in range(2):
                        cs = half * 1024 + nb * 512
                        for ki in range(8):
                            nc.tensor.matmul(
                                ps[:, nb * 512:nb * 512 + 512],
                                lhsT=w[:, ki, :],
                                rhs=xT_sb[:, ki, cs:cs + 512],
                                start=(ki == 0),
                                stop=(ki == 7 and not with_bias),
                            )
                        if with_bias:
                            nc.tensor.matmul(
                                ps[:, nb * 512:nb * 512 + 512],
                                lhsT=bias[0:1, :],
                                rhs=ones512[0:1, :],
                                start=False,
                                stop=True,
                            )
                    nc.vector.tensor_copy(
                        dst[:, half * 1024:half * 1024 + 1024], ps
                    )

            project(w_sb["v"], b_sb.get("bv"), vT_sb, "v")
            project(w_sb["k"], b_sb.get("bk"), kT_sb, "k")
            project(w_sb["q"], b_sb.get("bq"), qT_sb, "q")

            # ---- transpose v into [token, feat] blocks with ones column ----
            v_ones = sg.tile([128, B * KC, HPC, DH + 1], BF16, name="v_ones")
            nc.vector.memset(v_ones, 1.0)
            for tb in range(B * KC):
                pt = psB.tile([128, 128], BF16, tag="ctx", name=f"vt_{tb}")
                nc.tensor.transpose(pt, vT_sb[:, tb * 128:tb * 128 + 128], ident)
                for h in range(HPC):
                    nc.vector.tensor_copy(
                        v_ones[:, tb, h, 0:DH], pt[:, h * 64:h * 64 + 64]
                    )

            # ---- attention per (head, batch) + split AllGathers -----------
            ctxT_sb, ctxT_free = tc.tile([64, HPC, T], BF16, name="ctxT_sb")
            sums_r = sg.tile([1, S], F32R, name="sums_r")
            rec_sb = sg.tile([64, 1024], F32, name="rec_sb")

            cc_in = {}
            cc_out = {}
            for h in range(HPC):
                for b in range(B):
                    cc_in[(h, b)] = dram.tile([DH, S], BF16, name=f"cci_{h}_{b}")
                    cc_out[(h, b)] = dram.tile(
                        [NCORES * DH, S], BF16, addr_space="Shared",
                        name=f"cco_{h}_{b}",
                    )

            def normalize_qb(h, b, qb, ctx_tile):
                t0 = b * S
                j = qb % 2
                nc.vector.tensor_copy(
                    sums_r[0:1, qb * 512:qb * 512 + 512],
                    ctx_tile[DH:DH + 1, :],
                )
                bc = psA.tile([128, 512], F32, tag="work", name=f"bc_{h}_{b}_{qb}")
                nc.tensor.matmul(
                    bc[0:64, :],
                    lhsT=ones64r[0:1, :],
                    rhs=sums_r[0:1, qb * 512:qb * 512 + 512],
                    start=True,
                    stop=True,
                )
                nc.vector.reciprocal(rec_sb[:, j * 512:j * 512 + 512], bc[0:64, :])
                nc.vector.tensor_mul(
                    ctxT_sb[:, h, t0 + qb * 512:t0 + qb * 512 + 512],
                    ctx_tile[0:DH, :],
                    rec_sb[:, j * 512:j * 512 + 512],
                )

            for h in range(HPC):
                po = 64 * h
                for b in range(B):
                    t0 = b * S
                    ctx_ps = [
                        psB.tile([128, 512], F32, tag="ctx", name=f"ctx_{h}_{b}_{qb}")
                        for qb in range(QB)
                    ]
                    for kc in range(KC):
                        q0 = kc * 128  # batch-local
                        kT_blk = kT_sb[po:po + 64, t0 + q0:t0 + q0 + 128]
                        for hb in (0, 1024):
                            if hb + 1024 <= q0:
                                continue
                            lo = max(q0, hb)
                            hi = hb + 1024
                            st = psA.tile(
                                [128, 1024], F32, tag="work",
                                name=f"st_{h}_{b}_{kc}_{hb}",
                            )
                            c = lo
                            while c < hi:
                                c2 = min(hi, (c // 512 + 1) * 512)
                                nc.tensor.matmul(
                                    st[:, c - hb:c2 - hb],
                                    lhsT=kT_blk,
                                    rhs=qT_sb[po:po + 64, t0 + c:t0 + c2],
                                    start=True,
                                    stop=True,
                                )
                                c = c2
                            at = att_pool.tile([128, 1024], BF16, tag="att")
                            nc.scalar.activation(
                                at[:, lo - hb:hi - hb],
                                st[:, lo - hb:hi - hb],
                                Exp,
                                bias=maskb_sb[:, b * KC + kc:b * KC + kc + 1],
                                scale=0.125,
                            )
                            if lo == q0:  # diagonal 128-block
                                nc.vector.tensor_mul(
                                    at[:, q0 - hb:q0 - hb + 128],
                                    at[:, q0 - hb:q0 - hb + 128],
                                    trimask,
                                )
                            c = lo
                            while c < hi:
                                qb = c // 512
                                c2 = min(hi, (qb + 1) * 512)
                                nc.tensor.matmul(
                                    ctx_ps[qb][0:DH + 1, c - qb * 512:c2 - qb * 512],
                                    lhsT=v_ones[:, b * KC + kc, h, :],
                                    rhs=at[:, c - hb:c2 - hb],
                                    start=(kc == 0),
                                    stop=(kc == 4 * qb + 3),
                                )
                                c = c2
                        # normalize each q-block as soon as its k-loop is done
                        qb_done = (kc - 3) // 4
                        if kc >= 3 and (kc - 3) % 4 == 0:
                            normalize_qb(h, b, qb_done, ctx_ps[qb_done])
                    # ship this (head, batch) shard
                    nc.sync.dma_start(cc_in[(h, b)], ctxT_sb[:, h, t0:t0 + S])
                    nc.gpsimd.collective_compute(
                        "AllGather",
                        mybir.AluOpType.bypass,
                        replica_groups=[list(range(NCORES))],
                        ins=[cc_in[(h, b)].opt()],
                        outs=[cc_out[(h, b)].opt()],
                    )

            # free the big stage-B tiles before staging the gathered context
            # (LIFO: Tile singles are stack-allocated)
            ctxT_free()
            vT_free()
            kT_free()
            qT_free()
            xT_free()

            # gathered context: per (h, b) 4 chunks of 128 rows
            ctxF = {}
            ctxF_frees = []
            for h in range(HPC):
                for b in range(B):
                    t_, f_ = tc.tile([128, 4, S], BF16, name=f"ctxF_{h}_{b}")
                    ctxF[(h, b)] = t_
                    ctxF_frees.append(f_)
                    r = cc_out[(h, b)].rearrange("(o p) f -> p o f", p=128)
                    for ki in range(4):
                        nc.sync.dma_start(t_[:, ki, :], r[:, ki, :])

            # ---- out projection (column shard) ----------------------------
            outT_sb, outT_free = tc.tile([128, T], F32, name="outT_sb")
            for b in range(B):
                for half in range(2):
                    ps = psA.tile([128, 1024], F32, tag="work", name=f"o_{b}_{half}")
                    for nb in range(2):
                        cs = half * 1024 + nb * 512
                        first, last = (0, 0), (HPC - 1, 3)
                        for h in range(HPC):
                            for ki in range(4):
                                nc.tensor.matmul(
                                    ps[:, nb * 512:nb * 512 + 512],
                                    lhsT=w_sb["o"][:, h * 4 + ki, :],
                                    rhs=ctxF[(h, b)][:, ki, cs:cs + 512],
                                    start=((h, ki) == first),
                                    stop=((h, ki) == last and not with_bias),
                                )
                        if with_bias:
                            nc.tensor.matmul(
                                ps[:, nb * 512:nb * 512 + 512],
                                lhsT=b_sb["bo"][0:1, :],
                                rhs=ones512[0:1, :],
                                start=False,
                                stop=True,
                            )
                    cs0 = b * S + half * 1024
                    nc.vector.tensor_copy(outT_sb[:, cs0:cs0 + 1024], ps)
                    nc.sync.dma_start(
                        outT_d[:, cs0:cs0 + 1024], outT_sb[:, cs0:cs0 + 1024]
                    )
            outT_free()
            for f_ in reversed(ctxF_frees):
                f_()

    nc.compile()
    return nc


def _get_program(with_bias: bool = False):
    key = ("nc", with_bias)
    if key not in _CACHED:
        _CACHED[key] = _build(with_bias)
    return _CACHED[key]


# gathered feature order: [local-head h, core r, dh] -> global feature
# global head of (r, h) is 2r + h, so feature index = (2r + h) * DH + dh
_PERM = np.array(
    [(2 * r + h) * DH + dh for h in range(HPC) for r in range(NCORES) for dh in range(DH)]
)


def kernel(x, mask, wq, bq, wk, bk, wv, bv, wo, bo):
    x = np.asarray(x, dtype=np.float32)
    mask = np.asarray(mask)
    bf = ml_dtypes.bfloat16

    with_bias = any(np.any(np.asarray(bb)) for bb in (bq, bk, bv, bo))
    nc = _get_program(with_bias)

    # [feature, batch*seq] activations
    xT = np.ascontiguousarray(x.reshape(T, D).T).astype(bf)
    maskb = np.ascontiguousarray(
        np.where(np.asarray(mask).reshape(B * KC, 128), -10000.0, 0.0)
        .astype(np.float32)
        .T
    )
    in_maps = []
    for c in range(NCORES):
        fs = slice(c * F, (c + 1) * F)
        m = {
            "xT": xT,
            "wqT": np.ascontiguousarray(np.asarray(wq)[fs, :].T).astype(bf),
            "wkT": np.ascontiguousarray(np.asarray(wk)[fs, :].T).astype(bf),
            "wvT": np.ascontiguousarray(np.asarray(wv)[fs, :].T).astype(bf),
            "woT": np.ascontiguousarray(
                np.asarray(wo)[fs, :].T[_PERM]
            ).astype(bf),
            "maskb": maskb,
        }
        if with_bias:
            m["bq"] = np.asarray(bq)[fs].astype(bf).reshape(1, F)
            m["bk"] = np.asarray(bk)[fs].astype(bf).reshape(1, F)
            m["bv"] = np.asarray(bv)[fs].astype(bf).reshape(1, F)
            m["bo"] = np.asarray(bo)[fs].astype(bf).reshape(1, F)
        in_maps.append(m)

    res = bass_utils.run_bass_kernel_spmd(
        nc, in_maps, core_ids=list(range(NCORES)), trace=False
    )
    _CACHED["last_results"] = res

    out = np.empty((B, S, D), dtype=np.float32)
    for c in range(NCORES):
        o = res.results[c]["outT"]  # [F, T]
        out[:, :, c * F:(c + 1) * F] = o.T.reshape(B, S, F)
    return out


# revision 15
# speedup vs baseline: 1.2288x; 1.0115x over previous
"""Tensor-parallel multi-head attention for 8 Trainium2 NeuronCores.

Sharding (TP8 over heads): core c owns heads {2c, 2c+1} (128 q/k/v features)
and computes them for BOTH batch elements; out_proj is column-sharded with
8-core mesh AllGathers of the per-core context shards, split by (local head,
batch) so all but the last overlap remaining attention work.

Per-core dataflow (activations kept transposed, [feature, token]):
  qT/kT/vT = W.T-chunks @ xT          (PE, bf16, fp32 PSUM accum)
  v        = PE-transpose(vT)          (with an appended ones-column)
  sT[k,q]  = kT-block.T @ qT           (causal: upper-right blocks skipped)
  aT       = exp(sT/8 + mask_bias)     (ACT; safe without max-subtraction:
                                        scores ~ N(0,1))
  ctxT;sum = [v|1].T @ aT              (ones row gives the softmax denom)
  ctxT    *= 1/sum                     (per-q-block, as soon as its k-loop
                                        completes)
  AllGather ctxT shard per (head,batch), out-proj quarter per (batch,half)

Attention is emitted as an interleaved stream of two (head, batch) pairs —
each pair processes q-blocks {0,1} (k-chunks 0-7) then {2,3} (k-chunks 0-15)
— so the Tensor engine always has independent matmuls to run while the
Scalar engine works through the exp()s; without this the PE idles in
sub-3.5us slices every k-chunk and the HAM clock gate halves its clock.
Host side only reshapes/concatenates shards (dtype prep of inputs aside).
The gathered feature order is [local-head, core, dh]; the host permutes
wo's input dimension to match.
"""

import sys

for _p in ("/opt/trn_rl_repo",):
    if _p not in sys.path:
        sys.path.append(_p)

import numpy as np
import ml_dtypes

import concourse.bass as bass  # noqa: F401
import concourse.mybir as mybir
import concourse.tile as tile
from concourse import bacc, bass_utils
from concourse.masks import make_identity, make_upper_triangular

BF16 = mybir.dt.bfloat16
F32 = mybir.dt.float32
F32R = mybir.dt.float32r
Exp = mybir.ActivationFunctionType.Exp

B, S, D = 2, 2048, 1024
T = B * S            # 4096 tokens across batches
H, DH = 16, 64
NCORES = 8
HPC = H // NCORES    # heads per core = 2
F = HPC * DH         # features per core = 128
KC = S // 128        # 16 k-chunks per batch
QB = S // 512        # 4 q-blocks of 512 per batch

# attention pair order: (h, b); the last pair's AllGather is split in two
PAIRS = [(0, 0), (1, 0), (0, 1), (1, 1)]

_CACHED = {}


def _build(with_bias: bool):
    nc = bacc.Bacc(
        "TRN2",
        target_bir_lowering=False,
        debug=False,
        enable_asserts=True,
        num_devices=NCORES,
    )
    xT_d = nc.dram_tensor("xT", [D, T], BF16, kind="ExternalInput").ap()
    wqT_d = nc.dram_tensor("wqT", [D, F], BF16, kind="ExternalInput").ap()
    wkT_d = nc.dram_tensor("wkT", [D, F], BF16, kind="ExternalInput").ap()
    wvT_d = nc.dram_tensor("wvT", [D, F], BF16, kind="ExternalInput").ap()
    woT_d = nc.dram_tensor("woT", [D, F], BF16, kind="ExternalInput").ap()
    b_d = {}
    if with_bias:
        for nm in ("bq", "bk", "bv", "bo"):
            b_d[nm] = nc.dram_tensor(nm, [1, F], BF16, kind="ExternalInput").ap()
    maskb_d = nc.dram_tensor("maskb", [128, B * KC], F32, kind="ExternalInput").ap()
    outT_d = nc.dram_tensor("outT", [F, T], F32, kind="ExternalOutput").ap()

    with tile.TileContext(nc) as tc:
        with (
            tc.tile_pool(name="singles", bufs=1) as sg,
            tc.tile_pool(name="att", bufs=4) as att_pool,
            tc.tile_pool(name="psA", bufs=2, space="PSUM") as psA,
            tc.tile_pool(name="psB", bufs=4, space="PSUM") as psB,
            tc.tile_pool(name="dram", bufs=1, space="DRAM") as dram,
        ):
            # ---- constants -------------------------------------------------
            ident = sg.tile([128, 128], BF16, name="ident")
            make_identity(nc, ident)
            trimask = sg.tile([128, 128], BF16, name="trimask")
            make_upper_triangular(nc, trimask, val=1.0, diag=True)
            ones64f = sg.tile([1, 64], F32, name="ones64f")
            nc.vector.memset(ones64f, 1.0)
            ones64r = sg.tile([1, 64], F32R, name="ones64r")
            nc.vector.tensor_copy(ones64r, ones64f)
            if with_bias:
                ones512 = sg.tile([1, 512], BF16, name="ones512")
                nc.vector.memset(ones512, 1.0)

            # ---- load inputs (split for early start) -----------------------
            maskb_sb = sg.tile([128, B * KC], F32, name="maskb_sb")
            nc.sync.dma_start(maskb_sb, maskb_d)
            w_sb = {}
            for nm, dd in (("v", wvT_d), ("k", wkT_d), ("q", wqT_d), ("o", woT_d)):
                w_sb[nm] = sg.tile([128, 8, F], BF16, name=f"w{nm}T_sb")
                nc.sync.dma_start(w_sb[nm], dd.rearrange("(o p) f -> p o f", p=128))
            b_sb = {}
            if with_bias:
                for nm in ("bq", "bk", "bv", "bo"):
                    b_sb[nm] = sg.tile([1, F], BF16, name=f"{nm}_sb")
                    nc.sync.dma_start(b_sb[nm], b_d[nm])

            # persistent activations first, xT last (freed first: LIFO stack)
            qT_sb, qT_free = tc.tile([128, T], BF16, name="qT_sb")
            kT_sb, kT_free = tc.tile([128, T], BF16, name="kT_sb")
            ctxT_sb, ctxT_free = tc.tile([64, HPC, T], BF16, name="ctxT_sb")
            vT_sb, vT_free = tc.tile([128, T], BF16, name="vT_sb")
            xT_sb, xT_free = tc.tile([128, 8, T], BF16, name="xT_sb")
            xT_r = xT_d.rearrange("(o p) f -> p o f", p=128)
            for half in range(4):
                for ki in range(8):
                    cs = half * 1024
                    nc.sync.dma_start(
                        xT_sb[:, ki, cs:cs + 1024], xT_r[:, ki, cs:cs + 1024]
                    )

            # ---- projections ----------------------------------------------
            def project(w, bias, dst, which):
                for half in range(4):
                    ps = psA.tile(
                        [128, 1024], F32, tag="work", name=f"p_{which}_{half}"
                    )
                    for nb in range(2):
                        cs = half * 1024 + nb * 512
                        for ki in range(8):
                            nc.tensor.matmul(
                                ps[:, nb * 512:nb * 512 + 512],
                                lhsT=w[:, ki, :],
                                rhs=xT_sb[:, ki, cs:cs + 512],
                                start=(ki == 0),
                                stop=(ki == 7 and not with_bias),
                            )
                        if with_bias:
                            nc.tensor.matmul(
                                ps[:, nb * 512:nb * 512 + 512],
                                lhsT=bias[0:1, :],
                                rhs=ones512[0:1, :],
                                start=False,
                                stop=True,
                            )
                    nc.vector.tensor_copy(
                        dst[:, half * 1024:half * 1024 + 1024], ps
                    )

            project(w_sb["v"], b_sb.get("bv"), vT_sb, "v")
            project(w_sb["k"], b_sb.get("bk"), kT_sb, "k")
            project(w_sb["q"], b_sb.get("bq"), qT_sb, "q")

            # ---- transpose v into [token, feat] blocks with ones column ----
            v_ones = sg.tile([128, B * KC, HPC, DH + 1], BF16, name="v_ones")
            nc.vector.memset(v_ones, 1.0)
            for tb in range(B * KC):
                pt = psB.tile([128, 128], BF16, tag="ctx", name=f"vt_{tb}")
                nc.tensor.transpose(pt, vT_sb[:, tb * 128:tb * 128 + 128], ident)
                for h in range(HPC):
                    nc.vector.tensor_copy(
                        v_ones[:, tb, h, 0:DH], pt[:, h * 64:h * 64 + 64]
                    )

            # ---- attention: interleaved (head, batch) pair streams --------
            sums_r = sg.tile([1, 2 * S], F32R, name="sums_r")
            rec_sb = sg.tile([64, 2048], F32, name="rec_sb")

            cc_in = {}
            cc_out = {}
            for h, b in PAIRS[:-1]:
                cc_in[(h, b)] = dram.tile([DH, S], BF16, name=f"cci_{h}_{b}")
                cc_out[(h, b)] = dram.tile(
                    [NCORES * DH, S], BF16, addr_space="Shared", name=f"cco_{h}_{b}"
                )
            hL, bL = PAIRS[-1]
            for half in range(2):
                cc_in[(hL, bL, half)] = dram.tile(
                    [DH, 1024], BF16, name=f"cci_L_{half}"
                )
                cc_out[(hL, bL, half)] = dram.tile(
                    [NCORES * DH, 1024], BF16, addr_space="Shared",
                    name=f"cco_L_{half}",
                )

            def normalize_qb(h, b, lane, qb, ctx_tile):
                t0 = b * S
                so = lane * S + qb * 512
                ro = lane * 1024 + (qb % 2) * 512
                nc.vector.tensor_copy(
                    sums_r[0:1, so:so + 512], ctx_tile[DH:DH + 1, :]
                )
                bc = psA.tile(
                    [128, 512], F32, tag="work", name=f"bc_{h}_{b}_{qb}"
                )
                nc.tensor.matmul(
                    bc[0:64, :],
                    lhsT=ones64r[0:1, :],
                    rhs=sums_r[0:1, so:so + 512],
                    start=True,
                    stop=True,
                )
                nc.vector.reciprocal(rec_sb[:, ro:ro + 512], bc[0:64, :])
                nc.vector.tensor_mul(
                    ctxT_sb[:, h, t0 + qb * 512:t0 + qb * 512 + 512],
                    ctx_tile[0:DH, :],
                    rec_sb[:, ro:ro + 512],
                )

            def kc_step(h, b, lane, kc, qlo, qhi, ctx_ps, kc0_is_start):
                """One k-chunk of one pass: scores -> exp -> (mask) -> ctx."""
                po = 64 * h
                t0 = b * S
                q0 = kc * 128
                lo = max(q0, qlo)
                kT_blk = kT_sb[po:po + 64, t0 + q0:t0 + q0 + 128]
                st = psA.tile(
                    [128, 1024], F32, tag="work", name=f"st_{h}_{b}_{kc}_{qlo}"
                )
                c = lo
                while c < qhi:
                    c2 = min(qhi, (c // 512 + 1) * 512)
                    nc.tensor.matmul(
                        st[:, c - qlo:c2 - qlo],
                        lhsT=kT_blk,
                        rhs=qT_sb[po:po + 64, t0 + c:t0 + c2],
                        start=True,
                        stop=True,
                    )
                    c = c2
                at = att_pool.tile([128, 1024], BF16, tag="att")
                nc.scalar.activation(
                    at[:, lo - qlo:qhi - qlo],
                    st[:, lo - qlo:qhi - qlo],
                    Exp,
                    bias=maskb_sb[:, b * KC + kc:b * KC + kc + 1],
                    scale=0.125,
                )
                if lo == q0:  # diagonal 128-block: causal interior
                    nc.vector.tensor_mul(
                        at[:, q0 - qlo:q0 - qlo + 128],
                        at[:, q0 - qlo:q0 - qlo + 128],
                        trimask,
                    )
                c = lo
                while c < qhi:
                    qb = c // 512
                    c2 = min(qhi, (qb + 1) * 512)
                    nc.tensor.matmul(
                        ctx_ps[qb][0:DH + 1, c - qb * 512:c2 - qb * 512],
                        lhsT=v_ones[:, b * KC + kc, h, :],
                        rhs=at[:, c - qlo:c2 - qlo],
                        start=kc0_is_start and (kc == 0),
                        stop=(kc == 4 * qb + 3),
                    )
                    c = c2
                # normalize any q-block whose k-loop just completed
                if kc >= 3 and (kc - 3) % 4 == 0:
                    qb_done = (kc - 3) // 4
                    if qlo <= qb_done * 512 < qhi:
                        normalize_qb(h, b, lane, qb_done, ctx_ps[qb_done])

            def ship(h, b, half=None):
                t0 = b * S
                if half is None:
                    key, cols = (h, b), slice(t0, t0 + S)
                else:
                    key, cols = (h, b, half), slice(
                        t0 + half * 1024, t0 + half * 1024 + 1024
                    )
                nc.sync.dma_start(cc_in[key], ctxT_sb[:, h, cols])
                nc.gpsimd.collective_compute(
                    "AllGather",
                    mybir.AluOpType.bypass,
                    replica_groups=[list(range(NCORES))],
                    ins=[cc_in[key].opt()],
                    outs=[cc_out[key].opt()],
                )

            def pair_steps(h, b, lane):
                """Two passes: q-blocks {0,1} over kc 0..7, then {2,3} over
                kc 0..15.  Returns (pass1_steps, pass2_steps) of closures."""
                ctx1 = [
                    psB.tile([128, 512], F32, tag="ctx", name=f"cx_{h}_{b}_{qb}")
                    for qb in range(2)
                ]
                ctx2 = [
                    psB.tile([128, 512], F32, tag="ctx", name=f"cx_{h}_{b}_{qb+2}")
                    for qb in range(2)
                ]
                ctx_lo = {0: ctx1[0], 1: ctx1[1]}
                ctx_hi = {2: ctx2[0], 3: ctx2[1]}
                p1 = [
                    (lambda kc=kc: kc_step(h, b, lane, kc, 0, 1024, ctx_lo, True))
                    for kc in range(8)
                ]
                p2 = [
                    (lambda kc=kc: kc_step(h, b, lane, kc, 1024, 2048, ctx_hi, True))
                    for kc in range(KC)
                ]
                return p1, p2

            # lazily create psum tiles at emission time via closure capture:
            # pair_steps allocates its ctx tiles when called, so call in order.
            plan = []
            lanes = {}
            for i, (h, b) in enumerate(PAIRS):
                lanes[(h, b)] = i % 2

            steps1 = {}
            steps2 = {}

            def get_steps(p):
                if p not in steps1:
                    h, b = p
                    steps1[p], steps2[p] = pair_steps(h, b, lanes[p])
                return steps1[p], steps2[p]

            # emission schedule: P0.pass1; then interleave Pi.pass2 (16) with
            # P(i+1).pass1 (8) at 2:1; ship AGs at pass completions.
            pA = PAIRS[0]
            a1, _ = get_steps(pA)
            for s in a1:
                s()
            for i in range(len(PAIRS)):
                p = PAIRS[i]
                _, a2 = get_steps(p)
                nxt = PAIRS[i + 1] if i + 1 < len(PAIRS) else None
                b1 = get_steps(nxt)[0] if nxt is not None else []
                bi = 0
                for j, s in enumerate(a2):
                    s()
                    if i == len(PAIRS) - 1 and j == 7:
                        # last pair: q-half 0 (blocks 0,1 of pass1... pass2
                        # covers 1024.. so half 0 complete after ITS pass1)
                        pass
                    if j % 2 == 1 and bi < len(b1):
                        b1[bi]()
                        bi += 1
                while bi < len(b1):
                    b1[bi]()
                    bi += 1
                h, b = p
                if i < len(PAIRS) - 1:
                    ship(h, b)
                else:
                    ship(h, b, half=1)
                if nxt is not None and i + 2 == len(PAIRS):
                    # after the second-to-last pair's pass2 is emitted, the
                    # last pair's pass1 (emitted interleaved above) is done:
                    # ship its first q-half early
                    ship(*nxt, half=0)

            # free xT (stack top) now that projections are done
            xT_free()
            vT_free()

            # gathered context chunks + out-projection quarters
            ctxF = {}
            ctxF_frees = []
            for h in range(HPC):
                for b in range(B):
                    t_, f_ = tc.tile([128, 4, S], BF16, name=f"ctxF_{h}_{b}")
                    ctxF[(h, b)] = t_
                    ctxF_frees.append(f_)
                    if (h, b) != (hL, bL):
                        r = cc_out[(h, b)].rearrange("(o p) f -> p o f", p=128)
                        for ki in range(4):
                            nc.sync.dma_start(t_[:, ki, :], r[:, ki, :])
                    else:
                        for half in range(2):
                            r = cc_out[(h, b, half)].rearrange(
                                "(o p) f -> p o f", p=128
                            )
                            for ki in range(4):
                                nc.sync.dma_start(
                                    t_[:, ki, half * 1024:half * 1024 + 1024],
                                    r[:, ki, :],
                                )

            outT_sb, outT_free = tc.tile([128, T], F32, name="outT_sb")
            for b in range(B):
                for half in range(2):
                    ps = psA.tile(
                        [128, 1024], F32, tag="work", name=f"o_{b}_{half}"
                    )
                    for nb in range(2):
                        cs = half * 1024 + nb * 512
                        first, last = (0, 0), (HPC - 1, 3)
                        for h in range(HPC):
                            for ki in range(4):
                                nc.tensor.matmul(
                                    ps[:, nb * 512:nb * 512 + 512],
                                    lhsT=w_sb["o"][:, h * 4 + ki, :],
                                    rhs=ctxF[(h, b)][:, ki, cs:cs + 512],
                                    start=((h, ki) == first),
                                    stop=((h, ki) == last and not with_bias),
                                )
                        if with_bias:
                            nc.tensor.matmul(
                                ps[:, nb * 512:nb * 512 + 512],
                                lhsT=b_sb["bo"][0:1, :],
                                rhs=ones512[0:1, :],
                                start=False,
                                stop=True,
                            )
                    cs0 = b * S + half * 1024
                    nc.vector.tensor_copy(outT_sb[:, cs0:cs0 + 1024], ps)
                    nc.sync.dma_start(
                        outT_d[:, cs0:cs0 + 1024], outT_sb[:, cs0:cs0 + 1024]
                    )
            outT_free()
            for f_ in reversed(ctxF_frees):
                f_()
            ctxT_free()
            kT_free()
            qT_free()

    nc.compile()
    return nc


def _get_program(with_bias: bool = False):
    key = ("nc", with_bias)
    if key not in _CACHED:
        _CACHED[key] = _build(with_bias)
    return _CACHED[key]


# gathered feature order: [local-head h, core r, dh] -> global feature
# global head of (r, h) is 2r + h, so feature index = (2r + h) * DH + dh
_PERM = np.array(
    [(2 * r + h) * DH + dh for h in range(HPC) for r in range(NCORES) for dh in range(DH)]
)


def kernel(x, mask, wq, bq, wk, bk, wv, bv, wo, bo):
    x = np.asarray(x, dtype=np.float32)
    mask = np.asarray(mask)
    bf = ml_dtypes.bfloat16

    with_bias = any(np.any(np.asarray(bb)) for bb in (bq, bk, bv, bo))
    nc = _get_program(with_bias)

    # [feature, batch*seq] activations
    xT = np.ascontiguousarray(x.reshape(T, D).T).astype(bf)
    maskb = np.ascontiguousarray(
        np.where(np.asarray(mask).reshape(B * KC, 128), -10000.0, 0.0)
        .astype(np.float32)
        .T
    )
    in_maps = []
    for c in range(NCORES):
        fs = slice(c * F, (c + 1) * F)
        m = {
            "xT": xT,
            "wqT": np.ascontiguousarray(np.asarray(wq)[fs, :].T).astype(bf),
            "wkT": np.ascontiguousarray(np.asarray(wk)[fs, :].T).astype(bf),
            "wvT": np.ascontiguousarray(np.asarray(wv)[fs, :].T).astype(bf),
            "woT": np.ascontiguousarray(
                np.asarray(wo)[fs, :].T[_PERM]
            ).astype(bf),
            "maskb": maskb,
        }
        if with_bias:
            m["bq"] = np.asarray(bq)[fs].astype(bf).reshape(1, F)
            m["bk"] = np.asarray(bk)[fs].astype(bf).reshape(1, F)
            m["bv"] = np.asarray(bv)[fs].astype(bf).reshape(1, F)
            m["bo"] = np.asarray(bo)[fs].astype(bf).reshape(1, F)
        in_maps.append(m)

    res = bass_utils.run_bass_kernel_spmd(
        nc, in_maps, core_ids=list(range(NCORES)), trace=False
    )
    _CACHED["last_results"] = res

    out = np.empty((B, S, D), dtype=np.float32)
    for c in range(NCORES):
        o = res.results[c]["outT"]  # [F, T]
        out[:, :, c * F:(c + 1) * F] = o.T.reshape(B, S, F)
    return out
